# revision 1
# baseline (speedup 1.0000x reference)
"""Trainium2 Bass kernel for nn_BiLSTMDecoderModel.

Strategy (8 NeuronCores, data-parallel over batch, B=128 -> 16 rows/core):
  * backward LSTM: only b_hs[0] is consumed downstream == ONE cell step on x_0.
  * forward LSTM: 256-step scan. Recurrent matmul uses h-stationary layout
    (LDWEIGHTS cost ~ M=16 cols) with 4-way PE column tiling so the four
    512-col gate blocks stream concurrently through 4 XBUSes.
  * all gate nonlinearities collapse to tanh via sigmoid(x)=(tanh(x/2)+1)/2;
    the 1/2 factors are folded into host-preprocessed weights. The carried
    cell state is z=2c and hidden is h'=2h (Whh pre-scaled by 0.5).
  * per-step gate math runs in hidden-on-partitions layout (after 4 PE
    transposes), which directly yields the transposed-h stationary for the
    next step's matmuls.
  * input projections for all timesteps are precomputed in one batched
    matmul phase and streamed back per-step from DRAM; they enter the PSUM
    accumulation via an identity-stationary matmul (adds bias too).
"""

import sys

sys.path.insert(0, "/opt/trn_rl_repo")

import numpy as np
import ml_dtypes

import concourse.bass as bass
import concourse.mybir as mybir
import concourse.tile as tile
from concourse import bacc
from concourse.bass_utils import run_bass_kernel_spmd
from concourse.masks import make_identity

V, E, H, NCLS = 100000, 300, 512, 6
B, S = 128, 256
NC = 8
BL = B // NC  # 16
G4 = 4 * H  # 2048
G3 = 3 * 2 * H  # 3072
H2 = 2 * H  # 1024

f32 = mybir.dt.float32
bf16 = mybir.dt.bfloat16
i32 = mybir.dt.int32
Tanh = mybir.ActivationFunctionType.Tanh
Exp = mybir.ActivationFunctionType.Exp
Ln = mybir.ActivationFunctionType.Ln
Ident = mybir.ActivationFunctionType.Identity
ADD = mybir.AluOpType.add
SUB = mybir.AluOpType.subtract
MUL = mybir.AluOpType.mult
MAX = mybir.AluOpType.max

_cache = {}


def _bf(x):
    return np.ascontiguousarray(x.astype(ml_dtypes.bfloat16))


def _build_program():
    nc = bacc.Bacc(
        "TRN2", target_bir_lowering=False, debug=False, enable_asserts=False,
        num_devices=NC,
    )
    # ---- DRAM I/O ----
    embedW_d = nc.dram_tensor("embedW", [V, E], f32, kind="ExternalInput").ap()
    idx_d = nc.dram_tensor("idx", [128, 32], i32, kind="ExternalInput").ap()
    wihT_d = nc.dram_tensor("wihT", [304, G4], bf16, kind="ExternalInput").ap()
    bwihT_d = nc.dram_tensor("bwihT", [304, G4], bf16, kind="ExternalInput").ap()
    whhT_d = nc.dram_tensor("whhT", [H, G4], bf16, kind="ExternalInput").ap()
    dwhhT_d = nc.dram_tensor("dwhhT", [1028, G3], bf16, kind="ExternalInput").ap()
    dwihT_d = nc.dram_tensor("dwihT", [516, G3], bf16, kind="ExternalInput").ap()
    ecw_d = nc.dram_tensor("ecw", [NCLS, H], f32, kind="ExternalInput").ap()
    clsT_d = nc.dram_tensor("clsT", [1028, 2], bf16, kind="ExternalInput").ap()
    out_d = nc.dram_tensor("out", [NCLS, BL, 2], f32, kind="ExternalOutput").ap()

    with tile.TileContext(nc) as tc:
        _emit(nc, tc, embedW_d, idx_d, wihT_d, bwihT_d, whhT_d, dwhhT_d,
              dwihT_d, ecw_d, clsT_d, out_d)
    nc.compile()
    return nc


def _emit(nc, tc, embedW_d, idx_d, wihT_d, bwihT_d, whhT_d, dwhhT_d, dwihT_d,
          ecw_d, clsT_d, out_d):
    ctx_pools = []

    def pool(**kw):
        return tc.alloc_tile_pool(**kw)

    const = pool(name="const", bufs=1)
    dramp = pool(name="dram", bufs=1, space="DRAM")

    # ---- persistent SBUF constants ----
    ident = const.tile([128, 128], f32, tag="ident", name="ident")
    make_identity(nc, ident[:])
    identb = const.tile([128, 128], bf16, tag="identb", name="identb")
    make_identity(nc, identb[:])
    i16 = const.tile([16, 16], bf16, tag="i16", name="i16")
    make_identity(nc, i16[:])
    ones1 = const.tile([1, 16], bf16, tag="ones1", name="ones1")
    nc.gpsimd.memset(ones1[:], 1.0)
    onesr = const.tile([1, 128], bf16, tag="onesr", name="onesr")
    nc.gpsimd.memset(onesr[:], 1.0)
    biasf = const.tile([1, 2048], bf16, tag="biasf", name="biasf")
    biasb = const.tile([1, 2048], bf16, tag="biasb", name="biasb")
    bias_stat = const.tile([4, 16], bf16, tag="bias_stat", name="bias_stat")
    nc.gpsimd.memset(bias_stat[:], 0.0)
    nc.gpsimd.memset(bias_stat[0:1, :], 1.0)


    def tr(out_ap, in_ap, pin):
        nc.tensor.transpose(out_ap, in_ap, ident[0:pin, 0:pin])

    idx_sb = const.tile([128, 32], i32, tag="idx", name="idx")
    nc.sync.dma_start(idx_sb[:], idx_d[:])

    wih_sb = [const.tile([128, G4], bf16, tag=f"wih{k}", name=f"wih{k}") for k in range(3)]
    bwih_sb = [const.tile([128, G4], bf16, tag=f"bwih{k}", name=f"bwih{k}") for k in range(3)]
    for k in range(2):
        nc.sync.dma_start(wih_sb[k][:], wihT_d[128 * k:128 * (k + 1), :])
        nc.sync.dma_start(bwih_sb[k][:], bwihT_d[128 * k:128 * (k + 1), :])
    nc.sync.dma_start(wih_sb[2][0:44, :], wihT_d[256:300, :])
    nc.sync.dma_start(bwih_sb[2][0:44, :], bwihT_d[256:300, :])
    nc.sync.dma_start(biasf[:], wihT_d[300:301, :])
    nc.sync.dma_start(biasb[:], bwihT_d[300:301, :])

    whh_sb = [const.tile([128, G4], bf16, tag=f"whh{k}", name=f"whh{k}") for k in range(4)]
    for k in range(4):
        nc.sync.dma_start(whh_sb[k][:], whhT_d[128 * k:128 * (k + 1), :])

    dwhh_sb = [const.tile([128, G3], bf16, tag=f"dwhh{k}", name=f"dwhh{k}") for k in range(9)]
    for k in range(8):
        nc.sync.dma_start(dwhh_sb[k][:], dwhhT_d[128 * k:128 * (k + 1), :])
    nc.sync.dma_start(dwhh_sb[8][0:4, :], dwhhT_d[1024:1028, :])

    dwih_sb = [const.tile([128, G3], bf16, tag=f"dwih{k}", name=f"dwih{k}") for k in range(5)]
    for k in range(4):
        nc.sync.dma_start(dwih_sb[k][:], dwihT_d[128 * k:128 * (k + 1), :])
    nc.sync.dma_start(dwih_sb[4][0:4, :], dwihT_d[512:516, :])

    cls_sb = [const.tile([128, 2], bf16, tag=f"cls{k}", name=f"cls{k}") for k in range(9)]
    for k in range(8):
        nc.sync.dma_start(cls_sb[k][:], clsT_d[128 * k:128 * (k + 1), :])
    nc.sync.dma_start(cls_sb[8][0:4, :], clsT_d[1024:1028, :])

    xproj_d = dramp.tile([S * BL, G4], bf16, tag="xproj", name="xproj")

    # state tiles that persist across phases
    bH = const.tile([128, 64], bf16, tag="bH", name="bH")  # backward-cell h' (2h), hidden-parts

    # ======== Phase A: gather + tanh + transpose + input projection ========
    pA = pool(name="pA", bufs=3)
    pAx = pool(name="pAx", bufs=2)
    pAps = pool(name="pAps", bufs=1, space="PSUM")
    pAps2 = pool(name="pAps2", bufs=2, space="PSUM")
    pApsb = pool(name="pApsb", bufs=1, space="PSUM")

    emb0_a = None
    emb0_b = None
    for m in range(32):
        g_t = pA.tile([128, 304], f32, tag="gath", name="gath")
        nc.gpsimd.indirect_dma_start(
            out=g_t[:, 0:E],
            out_offset=None,
            in_=embedW_d[:],
            in_offset=bass.IndirectOffsetOnAxis(ap=idx_sb[:, m:m + 1], axis=0),
        )
        th = pA.tile([128, 304], f32, tag="th", name="th")
        nc.scalar.activation(th[:, 0:E], g_t[:, 0:E], Tanh)
        pst = pAps2.tile([128, 384], f32, tag="pst", name="pst")
        tr(pst[0:128, 0:128], th[:, 0:128], 128)
        tr(pst[0:128, 128:256], th[:, 128:256], 128)
        tr(pst[0:44, 256:384], th[:, 256:300], 128)
        embT_a = pA.tile([128, 256], bf16, tag="embTa", name="embTa")
        nc.vector.tensor_copy(embT_a[:], pst[:, 0:256])
        embT_b = pA.tile([48, 128], bf16, tag="embTb", name="embTb")
        nc.vector.tensor_copy(embT_b[0:44, :], pst[0:44, 256:384])

        psx = pAps.tile([128, G4], f32, tag="psx", name="psx")
        for nb in range(4):
            nsl = slice(512 * nb, 512 * (nb + 1))
            nc.tensor.matmul(psx[:, nsl], embT_a[:, 0:128], wih_sb[0][:, nsl],
                             start=True, stop=False)
            nc.tensor.matmul(psx[:, nsl], embT_a[:, 128:256], wih_sb[1][:, nsl],
                             start=False, stop=False)
            nc.tensor.matmul(psx[:, nsl], embT_b[0:44, :], wih_sb[2][0:44, nsl],
                             start=False, stop=False)
            nc.tensor.matmul(psx[:, nsl], onesr[:, 0:128], biasf[0:1, nsl],
                             start=False, stop=True)
        xp_m = pAx.tile([128, G4], bf16, tag="xpm", name="xpm")
        nc.scalar.activation(xp_m[:], psx[:], Ident)
        nc.sync.dma_start(xproj_d[128 * m:128 * (m + 1), :], xp_m[:])

        if m == 0:
            emb0_a, emb0_b = embT_a, embT_b
            # -------- backward LSTM single cell on x_0 (h=c=0) --------
            bps = pApsb.tile([128, 512], f32, tag="bps", name="bps")
            for j in range(4):
                ns = slice(512 * j, 512 * (j + 1))
                o = bps[32 * j:32 * j + 16, :]
                tp = (0, 32 * j)
                nc.tensor.matmul(o, emb0_a[:, 0:16], bwih_sb[0][:, ns],
                                 start=True, stop=False, tile_position=tp)
                nc.tensor.matmul(o, emb0_a[:, 128:144], bwih_sb[1][:, ns],
                                 start=False, stop=False, tile_position=tp)
                nc.tensor.matmul(o, emb0_b[0:44, 0:16], bwih_sb[2][0:44, ns],
                                 start=False, stop=False, tile_position=tp)
                nc.tensor.matmul(o, onesr[:, 0:16], biasb[0:1, ns],
                                 start=False, stop=True, tile_position=tp)
            bT = pA.tile([128, 512], f32, tag="bT", name="bT")
            nc.scalar.activation(bT[0:112, :], bps[0:112, :], Tanh)
            bpt = pApsb.tile([128, 448], f32, tag="bpt", name="bpt")
            for k in range(4):
                tr(bpt[:, 112 * k:112 * (k + 1)], bT[0:112, 128 * k:128 * (k + 1)], 112)
            bv = bpt[:].rearrange("p (c w) -> p c w", w=112)
            btip = pA.tile([128, 64], f32, tag="btip", name="btip")
            nc.vector.tensor_scalar_add(
                out=btip[:].rearrange("p (c w) -> p c w", w=16),
                in0=bv[:, :, 0:16], scalar1=1.0)
            bzv = pA.tile([128, 64], f32, tag="bzv", name="bzv")
            zb = bzv[:].rearrange("p (c w) -> p c w", w=16)
            nc.vector.tensor_tensor(
                out=zb, in0=btip[:].rearrange("p (c w) -> p c w", w=16),
                in1=bv[:, :, 64:80], op=MUL)
            btc = pA.tile([128, 64], f32, tag="btc", name="btc")
            nc.scalar.activation(btc[:], bzv[:], Tanh, scale=0.5)
            nc.vector.scalar_tensor_tensor(
                out=bH[:].rearrange("p (c w) -> p c w", w=16),
                in0=bv[:, :, 96:112], scalar=1.0,
                in1=btc[:].rearrange("p (c w) -> p c w", w=16),
                op0=ADD, op1=MUL)

    pApsb.release()
    pAps2.release()
    pAps.release()
    pAx.release()
    pA.release()

    # ======== Phase C: forward scan, 256 steps ========
    pH = pool(name="pH", bufs=2)
    pXP = pool(name="pXP", bufs=6)
    pPS = pool(name="pPS", bufs=2, space="PSUM")
    pT = pool(name="pT", bufs=2)
    pZ = pool(name="pZ", bufs=2)
    pW = pool(name="pW", bufs=3)

    z_prev = pZ.tile([128, 64], f32, tag="z", name="z")
    H_prev = pH.tile([128, 64], bf16, tag="H", name="H")
    nc.vector.memset(z_prev[:], 0.0)
    nc.vector.memset(H_prev[:], 0.0)

    for t in range(S):
        xp_t = pXP.tile([16, G4], bf16, tag="xp", name="xp")
        nc.sync.dma_start(xp_t[:], xproj_d[BL * t:BL * (t + 1), :])

        psg = pPS.tile([128, 512], f32, tag="psg", name="psg")
        for kc in range(4):
            for j in range(4):
                nc.tensor.matmul(
                    psg[32 * j:32 * j + 16, :],
                    H_prev[:, 16 * kc:16 * (kc + 1)],
                    whh_sb[kc][:, 512 * j:512 * (j + 1)],
                    start=(kc == 0), stop=False, tile_position=(0, 32 * j))
        for j in range(4):
            nc.tensor.matmul(
                psg[32 * j:32 * j + 16, :], i16[:],
                xp_t[:, 512 * j:512 * (j + 1)],
                start=False, stop=True, tile_position=(0, 32 * j))

        T_t = pT.tile([128, 512], f32, tag="T", name="T")
        nc.scalar.activation(T_t[0:112, :], psg[0:112, :], Tanh)
        pstT = pPS.tile([128, 448], f32, tag="pstT", name="pstT")
        for k in range(4):
            tr(pstT[:, 112 * k:112 * (k + 1)], T_t[0:112, 128 * k:128 * (k + 1)], 112)
        Tv = pstT[:].rearrange("p (c w) -> p c w", w=112)
        ti, tf = Tv[:, :, 0:16], Tv[:, :, 32:48]
        tg, to = Tv[:, :, 64:80], Tv[:, :, 96:112]

        a_t = pW.tile([128, 64], f32, tag="a", name="a")
        v_t = pW.tile([128, 64], f32, tag="v", name="v")
        av = a_t[:].rearrange("p (c w) -> p c w", w=16)
        vv = v_t[:].rearrange("p (c w) -> p c w", w=16)
        zpv = z_prev[:].rearrange("p (c w) -> p c w", w=16)
        nc.vector.scalar_tensor_tensor(out=av, in0=tf, scalar=1.0, in1=zpv,
                                       op0=ADD, op1=MUL)
        tip = pW.tile([128, 64], f32, tag="tip", name="tip")
        tipv = tip[:].rearrange("p (c w) -> p c w", w=16)
        nc.vector.tensor_scalar_add(out=tipv, in0=ti, scalar1=1.0)
        nc.vector.tensor_tensor(out=vv, in0=tipv, in1=tg, op=MUL)
        z_new = pZ.tile([128, 64], f32, tag="z", name="z")
        nc.vector.scalar_tensor_tensor(out=z_new[:], in0=a_t[:], scalar=0.5,
                                       in1=v_t[:], op0=MUL, op1=ADD)
        tc_t = pW.tile([128, 64], f32, tag="tc", name="tc")
        nc.scalar.activation(tc_t[:], z_new[:], Tanh, scale=0.5)
        H_new = pH.tile([128, 64], bf16, tag="H", name="H")
        nc.vector.scalar_tensor_tensor(
            out=H_new[:].rearrange("p (c w) -> p c w", w=16),
            in0=to, scalar=1.0,
            in1=tc_t[:].rearrange("p (c w) -> p c w", w=16),
            op0=ADD, op1=MUL)
        z_prev, H_prev = z_new, H_new

    pW.release()
    pZ.release()
    pT.release()
    pPS.release()
    pXP.release()

    # ======== Phase D: decoder (6 GRU steps + logits + log_softmax) ========
    pD = pool(name="pD", bufs=1)
    pDgi = pool(name="pDgi", bufs=1, space="PSUM")

    ce_t = pD.tile([NCLS, H], f32, tag="ce", name="ce")
    nc.sync.dma_start(ce_t[:], ecw_d[:])
    ce2 = pD.tile([NCLS, H], f32, tag="ce2", name="ce2")
    nc.scalar.activation(ce2[:], ce_t[:], Tanh)
    psc = pDgi.tile([128, 24], f32, tag="psc", name="psc")
    for k in range(4):
        tr(psc[:, 6 * k:6 * (k + 1)], ce2[0:NCLS, 128 * k:128 * (k + 1)], NCLS)
    ceT = pD.tile([128, 24], bf16, tag="ceT", name="ceT")
    nc.vector.tensor_copy(ceT[:], psc[:])

    psgi = pDgi.tile([NCLS, G3], f32, tag="psgi", name="psgi")
    for ng in range(6):
        ns = slice(512 * ng, 512 * (ng + 1))
        for kc in range(4):
            nc.tensor.matmul(psgi[:, ns], ceT[:, 6 * kc:6 * (kc + 1)],
                             dwih_sb[kc][:, ns], start=(kc == 0), stop=False)
        nc.tensor.matmul(psgi[:, ns], bias_stat[0:4, 0:NCLS],
                         dwih_sb[4][0:4, ns], start=False, stop=True)
    gi_sb = pD.tile([NCLS, G3], bf16, tag="gi", name="gi")
    nc.scalar.activation(gi_sb[:], psgi[:], Ident)

    # transposed gi_n (per-partition bias for the n-gate), pre-scaled by 1.0
    psgT = pDgi.tile([128, 48], bf16, tag="psgT", name="psgT")
    for gc in range(8):
        nc.tensor.transpose(psgT[:, 6 * gc:6 * (gc + 1)], gi_sb[0:NCLS, 2048 + 128 * gc:2048 + 128 * (gc + 1)], identb[0:NCLS, 0:NCLS])
    giT = pD.tile([128, 48], f32, tag="giT", name="giT")
    nc.vector.tensor_copy(giT[:], psgT[:])
    # per-class gi rows at partition 0 (PE movers must start at partition 0)
    gi_row = pD.tile([1, NCLS * G3], bf16, tag="girow", name="girow")
    for c in range(NCLS):
        for ng in range(6):
            ns = slice(512 * ng, 512 * (ng + 1))
            for kc in range(4):
                nc.tensor.matmul(psgi[0:1, ns], ceT[:, 6 * kc + c:6 * kc + c + 1],
                                 dwih_sb[kc][:, ns], start=(kc == 0), stop=False)
            nc.tensor.matmul(psgi[0:1, ns], bias_stat[0:4, 0:1],
                             dwih_sb[4][0:4, ns], start=False, stop=True)
        nc.scalar.activation(gi_row[0:1, G3 * c:G3 * (c + 1)], psgi[0:1, :], Ident)
    pDgi.release()
    pDps = pool(name="pDps", bufs=1, space="PSUM")

    Hd = pD.tile([128, 128], bf16, tag="Hd", name="Hd")
    nc.vector.tensor_scalar_mul(Hd[:, 0:64], H_prev[:], 0.5)
    nc.vector.tensor_scalar_mul(Hd[:, 64:128], bH[:], 0.5)

    l_all = pD.tile([16, 12], f32, tag="lall", name="lall")

    for c in range(NCLS):
        psd0 = pDps.tile([128, 512], f32, tag="psd0", name="psd0")
        psd1 = pDps.tile([128, 512], f32, tag="psd1", name="psd1")
        for kc in range(8):
            lh = Hd[:, 16 * kc:16 * (kc + 1)]
            for ng in range(6):
                ps, j = (psd0, ng) if ng < 4 else (psd1, ng - 4)
                nc.tensor.matmul(
                    ps[32 * j:32 * j + 16, :], lh,
                    dwhh_sb[kc][:, 512 * ng:512 * (ng + 1)],
                    start=(kc == 0), stop=False, tile_position=(0, 32 * j))
        for ng in range(6):
            ps, j = (psd0, ng) if ng < 4 else (psd1, ng - 4)
            nc.tensor.matmul(
                ps[32 * j:32 * j + 16, :], bias_stat[0:4, :],
                dwhh_sb[8][0:4, 512 * ng:512 * (ng + 1)],
                start=False, stop=(ng >= 4), tile_position=(0, 32 * j))
        for ng in range(4):
            nc.tensor.matmul(
                psd0[32 * ng:32 * ng + 16, :], ones1[:],
                gi_row[0:1, G3 * c + 512 * ng:G3 * c + 512 * (ng + 1)],
                start=False, stop=True, tile_position=(0, 32 * ng))

        Trz = pD.tile([128, 512], f32, tag="Trz", name="Trz")
        nc.scalar.activation(Trz[0:112, :], psd0[0:112, :], Tanh)
        pstz = pDps.tile([128, 448], f32, tag="pstz", name="pstz")
        for k in range(4):
            tr(pstz[:, 112 * k:112 * (k + 1)], Trz[0:112, 128 * k:128 * (k + 1)], 112)
        hn_sb = pD.tile([48, 512], f32, tag="hn", name="hn")
        nc.vector.tensor_copy(hn_sb[:], psd1[0:48, :])
        psn = pDps.tile([128, 192], f32, tag="psn", name="psn")
        for k in range(4):
            tr(psn[:, 48 * k:48 * (k + 1)], hn_sb[0:48, 128 * k:128 * (k + 1)], 48)

        zv = pstz[:].rearrange("p (c w) -> p c w", w=112)
        nv = psn[:].rearrange("p (c w) -> p c w", w=48)
        trp = pD.tile([128, 128], f32, tag="trp", name="trp")
        trpv = trp[:].rearrange("p (g w) -> p g w", w=16)
        sn_t = pD.tile([128, 128], f32, tag="sn", name="sn")
        snv = sn_t[:].rearrange("p (g w) -> p g w", w=16)
        nT = pD.tile([128, 128], f32, tag="nT", name="nT")
        for s in range(2):
            nc.vector.tensor_scalar_add(
                out=trpv[:, 4 * s:4 * s + 4, :],
                in0=zv[:, :, 32 * s:32 * s + 16], scalar1=1.0)
            nc.vector.tensor_tensor(
                out=snv[:, 4 * s:4 * s + 4, :],
                in0=trpv[:, 4 * s:4 * s + 4, :],
                in1=nv[:, :, 32 * s:32 * s + 16], op=MUL)
        nTv = nT[:].rearrange("p (g w) -> p g w", w=16)
        for g in range(8):
            nc.scalar.activation(nTv[:, g, :], snv[:, g, :], Tanh, scale=0.5,
                                 bias=giT[:, 6 * g + c:6 * g + c + 1])
        d_t = pD.tile([128, 128], f32, tag="dt", name="dt")
        nc.vector.tensor_tensor(out=d_t[:], in0=Hd[:], in1=nT[:], op=SUB)
        e_t = pD.tile([128, 128], f32, tag="et", name="et")
        ev = e_t[:].rearrange("p (g w) -> p g w", w=16)
        dv = d_t[:].rearrange("p (g w) -> p g w", w=16)
        for s in range(2):
            nc.vector.scalar_tensor_tensor(
                out=ev[:, 4 * s:4 * s + 4, :],
                in0=zv[:, :, 64 + 32 * s:80 + 32 * s], scalar=1.0,
                in1=dv[:, 4 * s:4 * s + 4, :], op0=ADD, op1=MUL)
        hn2 = pD.tile([128, 128], f32, tag="hn2", name="hn2")
        nc.vector.scalar_tensor_tensor(out=hn2[:], in0=e_t[:], scalar=0.5,
                                       in1=nT[:], op0=MUL, op1=ADD)
        Hd_new = pD.tile([128, 128], bf16, tag="Hd", name="Hd")
        nc.scalar.activation(Hd_new[:], hn2[:], Tanh)

        psl = pDps.tile([16, 2], f32, tag="psl", name="psl")
        for kc in range(8):
            nc.tensor.matmul(psl[:], Hd_new[:, 16 * kc:16 * (kc + 1)],
                             cls_sb[kc][:, 0:2], start=(kc == 0), stop=False)
        nc.tensor.matmul(psl[:], bias_stat[0:4, :], cls_sb[8][0:4, 0:2],
                         start=False, stop=True)
        nc.vector.tensor_copy(l_all[:, 2 * c:2 * c + 2], psl[:])
        Hd = Hd_new

    la = l_all[:].rearrange("p (c t) -> p c t", t=2)
    mx = pD.tile([16, 6], f32, tag="mx", name="mx")
    nc.vector.tensor_tensor(out=mx[:].rearrange("p (c o) -> p c o", o=1),
                            in0=la[:, :, 0:1], in1=la[:, :, 1:2], op=MAX)
    d0 = pD.tile([16, 12], f32, tag="d0", name="d0")
    d0v = d0[:].rearrange("p (c t) -> p c t", t=2)
    mxb = mx[:].rearrange("p (c o) -> p c o", o=1).to_broadcast([16, 6, 2])
    nc.vector.tensor_tensor(out=d0v, in0=la, in1=mxb, op=SUB)
    ex = pD.tile([16, 12], f32, tag="ex", name="ex")
    nc.scalar.activation(ex[:], d0[:], Exp)
    se = pD.tile([16, 6], f32, tag="se", name="se")
    nc.vector.tensor_reduce(out=se[:].rearrange("p (c o) -> p c o", o=1),
                            in_=ex[:].rearrange("p (c t) -> p c t", t=2),
                            op=ADD, axis=mybir.AxisListType.X)
    ls = pD.tile([16, 6], f32, tag="ls", name="ls")
    nc.scalar.activation(ls[:], se[:], Ln)
    ov = pD.tile([16, 12], f32, tag="ov", name="ov")
    lsb = ls[:].rearrange("p (c o) -> p c o", o=1).to_broadcast([16, 6, 2])
    nc.vector.tensor_tensor(out=ov[:].rearrange("p (c t) -> p c t", t=2),
                            in0=d0v, in1=lsb, op=SUB)
    nc.sync.dma_start(out_d[:].rearrange("c b t -> b c t"),
                      ov[:].rearrange("p (c t) -> p c t", t=2))

    pDps.release()
    pD.release()
    pH.release()
    dramp.release()
    const.release()


def _prep_inputs(seq, classes, embed_W, embed_class_W, f_Wih, f_Whh, f_b,
                 b_Wih, b_Whh, b_b, d_Wih, d_Whh, d_bih, d_bhh, cls_W, cls_b):
    seq = np.asarray(seq)
    s4 = np.concatenate([np.full(H, 0.5), np.full(H, 0.5), np.ones(H),
                         np.full(H, 0.5)]).astype(np.float32)
    s3 = np.concatenate([np.full(H2, 0.5), np.full(H2, 0.5),
                         np.ones(H2)]).astype(np.float32)

    def padrows(a, rows):
        out = np.zeros((rows, a.shape[1]), np.float32)
        out[:a.shape[0]] = a
        return out

    wihT = padrows(np.concatenate(
        [(f_Wih * s4[:, None]).T, (f_b * s4)[None, :]], axis=0), 304)
    bwihT = padrows(np.concatenate(
        [(b_Wih * s4[:, None]).T, (b_b * s4)[None, :]], axis=0), 304)
    whhT = ((f_Whh * s4[:, None]) * 0.5).T.astype(np.float32)
    dwhhT = padrows(np.concatenate(
        [(d_Whh * s3[:, None]).T, (d_bhh * s3)[None, :]], axis=0), 1028)
    dwihT = padrows(np.concatenate(
        [(d_Wih * s3[:, None]).T, (d_bih * s3)[None, :]], axis=0), 516)
    clsT = padrows(np.concatenate(
        [np.asarray(cls_W, np.float32).T, np.asarray(cls_b, np.float32)[None, :]],
        axis=0), 1028)
    ecw = np.asarray(embed_class_W, np.float32)[np.asarray(classes)]

    shared = {
        "embedW": np.ascontiguousarray(np.asarray(embed_W, np.float32)),
        "wihT": _bf(wihT), "bwihT": _bf(bwihT), "whhT": _bf(whhT),
        "dwhhT": _bf(dwhhT), "dwihT": _bf(dwihT),
        "ecw": np.ascontiguousarray(ecw),
        "clsT": _bf(clsT),
    }
    in_maps = []
    for c in range(NC):
        tok = np.asarray(seq[BL * c:BL * (c + 1), :], np.int32)  # [16, 256]
        idx = np.ascontiguousarray(
            tok.T.reshape(S * BL).reshape(32, 128).T.astype(np.int32))
        m = dict(shared)
        m["idx"] = idx
        in_maps.append(m)
    return in_maps


def kernel(**inputs):
    if "nc" not in _cache:
        _cache["nc"] = _build_program()
    nc = _cache["nc"]
    in_maps = _prep_inputs(**inputs)
    import os
    trace = bool(int(os.environ.get("BK_TRACE", "0")))
    res = run_bass_kernel_spmd(nc, in_maps, core_ids=list(range(NC)),
                               trace=trace)
    _cache["last_result"] = res
    outs = [res.results[c]["out"] for c in range(NC)]
    return np.concatenate(outs, axis=1).astype(np.float32)

